# revision 39
# baseline (speedup 1.0000x reference)
"""CBOW (nn_CBOW_88991722373900) Trainium2 kernel.

Full-input contract: kernel(context_words[10,128000] f32, W_in[300,128000] f32,
W_out[128000,300] f32) -> softmax probabilities [128000] f32.

Strategy (8-way tensor/model parallel over the vocab dim V):
  - shard V into 8 chunks of 16000; each core holds its slice of both weight
    matrices, cast on host to fp8e4 (e4m3, +-240 range) with power-of-two
    scales folded out later - memory-bound problem, quarters HBM traffic vs
    f32 and halves it vs bf16
  - GEMM1: partial hidden[10,300] accumulated in PSUM over 125 v-chunks,
    fp8 DoubleRow perf mode (2 v-chunks per matmul, 2x PE throughput);
    C and N padded to 16/304 for the DoubleRow 16B step constraint
  - C-reduce (ones-matmul) -> AllGather(300 bf16) -> rank-sum (ones-matmul),
    exact f32 (1/(C*S1)) on the PSUM->SBUF copies -> full hidden
  - GEMM2 split across two engines in parallel:
      PE half  (v-blocks 0..74):   fp8 W_out col-blocks stationary (FWL),
               hidden col moving -> PSUM [128, 75]
      DVE half (v-blocks 75..124): grouped bf16 tensor_mul against a
               partition-broadcast hidden + segmented bf16 tensor_reduce
               (all-2-byte operands keep the DVE 2x mode) -> SBUF [128, 50]
    v mapped so partition p holds contiguous v = 125*p + b
  - softmax: exp on ScalarE with the 2^-12 w2-scale folded into the
    activation scale (no max subtraction: |logit| << 1), local sum via
    ones-matmul, AllGather(4B) for the global denominator, scale, DMA out
"""

import numpy as np
import ml_dtypes

import concourse.bass as bass
import concourse.mybir as mybir
from concourse import tile
from concourse.bass_utils import run_bass_kernel_spmd
from concourse.vector_clock import ScopedClock, VectorClock

V = 128000
N = 300
C = 10
W = 8              # cores
VL = V // W        # 16000 vocab per core
NJ = VL // 128     # 125 v-chunks for GEMM1
NB = VL // 128     # 125 v-blocks for GEMM2
CP = 16            # C padded for DoubleRow 16B step constraint
NP = 304           # N padded (must be mult of 16)
NCH = [(0, 128), (128, 128)]  # n-chunks for GEMM2 PE (n<256; 44-tail on DVE)
N2 = 256           # n columns handled by the PE lane
NQ = N - N2        # 44-column n-tail of the PE-lane blocks, done on DVE
PEB = 125          # all v-blocks on the PE lane (2-chunk matvec)
W2G = 25           # PE-lane v-blocks per w2 SBUF tile group
NG2 = (PEB + W2G - 1) // W2G
QG = 42            # PE-lane blocks per n-tail SBUF tile group
NGQ = (PEB + QG - 1) // QG

S1 = float(2 ** 16)   # host scale on W_in  (max .0028*65536 = 184 < 240)
S2 = float(2 ** 12)   # host scale on W_out (max .0577*4096 = 236 < 240)

BF16 = mybir.dt.bfloat16
F32 = mybir.dt.float32
FP8 = mybir.dt.float8e4
NP_BF16 = ml_dtypes.bfloat16
NP_FP8 = ml_dtypes.float8_e4m3

# w1/ctx chunk group sizes (even prefix sums so DoubleRow pairs never span
# a group boundary); first groups small so GEMM1 starts early
PROBE = False  # shared-memory probe (failed NEFF compile; keep off)

W1_GROUPS = [2, 4, 6, 8] + [10] * 10 + [5]
assert sum(W1_GROUPS) == NJ


def _patched_drain_and_barrier(self, tick_clock, wait_clock):
    """Tail-drain waits split into 1-wait NOPs (this walrus build's CTRL
    instructions only encode a single sync wait), and the trailing
    all-engine barrier after the semaphore clear dropped: engines halt
    right after, and the runtime only starts a new execution once every
    engine has halted."""
    vc = tick_clock.global_clock
    procs = [(p, vc[p]) for p in range(len(vc)) if vc[p] > 0]
    for i, (p, t) in enumerate(procs):
        pvc = VectorClock([0] * len(vc))
        pvc.require_at_least(p, t)
        nop_inst = self.nc.sync.nop(nofuse=True, hint=f"tail_wait_{i}")
        wait_clock.add_sem_waits(nop_inst.ins, ScopedClock({None: pvc}))
    self.nc.sync.drain()
    self.nc.all_engine_barrier(sem_only=True)
    assert self.sems is not None
    popped = self.nc._tile_sem_poison_stack.pop()
    assert popped is self._sem_poison
    self.nc.clear_and_free_semaphores(list(self.sems.allocated().values()))


tile.TileContext._drain_and_barrier = _patched_drain_and_barrier


def _split_multi_waits(nc):
    """This walrus build encodes at most ONE sync wait per instruction. Hoist
    excess waits onto same-engine NoOps inserted immediately before."""
    import bass_rust

    ctr = [0]

    def make_nop(engine, wait):
        ctr[0] += 1
        nop = mybir.InstNoOp(name=f"I-wsplit{ctr[0]}", engine=engine)
        nop.bass_nofuse = True
        nop.sync_info = bass_rust.SyncInfo(on_wait=[wait], on_update=[])
        nc.register_instruction(nop, overwrite=True)
        return nop

    for bb in nc.main_func.blocks:
        out = []
        for ins in bb.instructions:
            si = ins.sync_info
            if si is not None and si.on_wait and len(si.on_wait) > 1:
                waits = list(si.on_wait)
                for w in waits[:-1]:
                    out.append(make_nop(ins.engine, w))
                ins.sync_info = bass_rust.SyncInfo(
                    on_wait=[waits[-1]], on_update=list(si.on_update)
                )
            out.append(ins)
        bb.instructions = out


def build_kernel():
    nc = bass.Bass()

    ctxp = nc.dram_tensor("ctxp", [128, NJ * CP], FP8, kind="ExternalInput")
    # w1t packed partition-major on host: w1t[p, j*NP + n] = W_in[n, v0+128j+p]*S1
    w1t = nc.dram_tensor("w1t", [128, NJ * NP], FP8, kind="ExternalInput")
    # w2p: PE lane, w2p[n, 128b+p] = W_out[v0+125p+b, n]*S2, b<PEB, n<256
    w2p = nc.dram_tensor("w2p", [N2, PEB * 128], FP8, kind="ExternalInput")
    # w2q: n-tail of the PE-lane blocks (DVE), w2q[p, b*NQ+q] =
    #      W_out[v0+125p+b, 256+q]*S2
    w2q = nc.dram_tensor("w2q", [128, PEB * NQ], BF16, kind="ExternalInput")
    y_out = nc.dram_tensor("y", [128, NB], F32, kind="ExternalOutput")
    if PROBE:
        shp = nc.dram_tensor("shp", [1, 1], mybir.dt.int32, addr_space="Shared")
        dbg_out = nc.dram_tensor("dbg", [1, 1], mybir.dt.int32, kind="ExternalOutput")

    with tile.TileContext(nc) as tc:
        with (
            tc.tile_pool(name="const", bufs=1) as cpool,
            tc.tile_pool(name="scr", bufs=2) as spool,
            tc.tile_pool(name="psum", bufs=1, space="PSUM") as ppool,
            tc.tile_pool(name="dram", bufs=1, space="DRAM") as dpool,
        ):
            # ---- shared-memory probe: all cores race-write their rank+10
            #      into one Shared DRAM word; read back at the end. Shared
            #      HBM => every core sees the same winner; per-core HBM =>
            #      each core reads back its own rank. ----
            if PROBE:
                rank = nc.gpsimd.partition_id()
                rk_sb = cpool.tile([1, 1], mybir.dt.int32, tag="rk")
                rk10 = nc.gpsimd.alloc_register("rk10")
                nc.gpsimd.reg_add(rk10, rank, 10)
                nc.gpsimd.reg_save(rk_sb[:, :], rk10)
                nc.gpsimd.dma_start(shp[:, :], rk_sb[:, :])

            # ---- ctx staged in two pieces so the first GEMM1 matmul only
            #      waits on a 4KB transfer; the large piece goes first on
            #      the fast SP HWDGE ring ----
            CTX_SPLIT = W1_GROUPS[0]  # chunks in the first piece
            ctx_a = cpool.tile([128, CTX_SPLIT * CP], FP8, tag="ctxa")
            nc.gpsimd.dma_start(ctx_a[:, :], ctxp[:, 0:CTX_SPLIT * CP])
            ctx_b = cpool.tile([128, (NJ - CTX_SPLIT) * CP], FP8, tag="ctxb")
            nc.sync.dma_start(ctx_b[:, :], ctxp[:, CTX_SPLIT * CP:])

            def ctx_slice(j, nchunks):
                if j < CTX_SPLIT:
                    return ctx_a[:, j * CP:(j + nchunks) * CP]
                jo = j - CTX_SPLIT
                return ctx_b[:, jo * CP:(jo + nchunks) * CP]

            ones10 = cpool.tile([C, 1], BF16, tag="ones10")
            nc.vector.memset(ones10[:, :], 1.0)
            ones8 = cpool.tile([W, 1], BF16, tag="ones8")
            nc.vector.memset(ones8[:, :], 1.0)
            onescol = cpool.tile([1, 128], F32, tag="onescol")
            nc.vector.memset(onescol[:, :], 1.0)
            ones128 = cpool.tile([128, 1], F32, tag="ones128")
            nc.vector.memset(ones128[:, :], 1.0)
            ones8w = cpool.tile([W, 128], BF16, tag="ones8w")
            nc.vector.memset(ones8w[:, :], 1.0)

            # ---- w1 stream, alternating HWDGE rings (SP / ACT) ----
            rings = [nc.sync, nc.scalar]
            ri = 0
            w1_sb = []
            j0 = 0
            for g, nj in enumerate(W1_GROUPS):
                t = cpool.tile([128, nj * NP], FP8, tag=f"w1_{g}")
                rings[ri % 2].dma_start(t[:, :], w1t[:, j0 * NP:(j0 + nj) * NP])
                ri += 1
                w1_sb.append((t, j0, nj))
                j0 += nj

            # w2 streams right behind w1 on the same rings
            w2_sb = {}
            for g in range(NG2):
                b0 = g * W2G
                nb = min(W2G, PEB - b0)
                for i3, (off, kk) in enumerate(NCH):
                    t = cpool.tile([kk, nb * 128], FP8, tag=f"w2_{i3}_{g}")
                    rings[ri % 2].dma_start(
                        t[:, :], w2p[off:off + kk, b0 * 128:(b0 + nb) * 128]
                    )
                    ri += 1
                    w2_sb[(i3, g)] = t
            w2q_sb = []
            for g in range(NGQ):
                qb0 = g * QG
                nb = min(QG, PEB - qb0)
                t = cpool.tile([128, nb * NQ], BF16, tag=f"w2q_{g}")
                rings[ri % 2].dma_start(
                    t[:, :], w2q[:, qb0 * NQ:(qb0 + nb) * NQ]
                )
                ri += 1
                w2q_sb.append((t, qb0, nb))

            # ---- GEMM1: psum_h[c, n] += ctx_chunk^T x w1 tile, fp8
            #      DoubleRow (2 v-chunks of 128 per matmul) ----
            psum_h = ppool.tile([CP, NP], F32, tag="ph")
            for t, j0g, nj in w1_sb:
                lj = 0
                while lj < nj:
                    j = j0g + lj
                    if lj + 2 <= nj and j + 2 <= NJ:
                        lhsT = ctx_slice(j, 2).rearrange(
                            "q (two c) -> q two c", two=2
                        )
                        rhs = t[:, lj * NP:(lj + 2) * NP].rearrange(
                            "q (two n) -> q two n", two=2
                        )
                        nc.tensor.matmul(
                            psum_h[:, :], lhsT, rhs,
                            start=(j == 0), stop=(j + 2 == NJ),
                            perf_mode=mybir.MatmulPerfMode.DoubleRow,
                        )
                        lj += 2
                    else:
                        nc.tensor.matmul(
                            psum_h[:, :],
                            ctx_slice(j, 1),
                            t[:, lj * NP:(lj + 1) * NP],
                            start=(j == 0), stop=(j + 1 == NJ),
                        )
                        lj += 1

            # tiny dummy Exp in the idle window so the activation table is
            # resident before the real softmax exp
            warm = cpool.tile([1, 1], F32, tag="warm")
            nc.vector.memset(warm[:, :], 0.0)
            warm2 = cpool.tile([1, 1], F32, tag="warm2")
            nc.scalar.activation(
                warm2[:, :], warm[:, :], mybir.ActivationFunctionType.Exp
            )

            # ---- local C-reduce -> [1, 300] bf16 (small AllGather payload;
            #      collective latency here is floor-dominated) ----
            h10 = cpool.tile([C, N], BF16, tag="h10")
            nc.vector.tensor_copy(h10[:, :], psum_h[0:C, 0:N])
            psum_hl = ppool.tile([1, N], F32, tag="phl")
            nc.tensor.matmul(psum_hl[:, :], ones10[:, :], h10[:, :])
            h_loc = cpool.tile([1, N], BF16, tag="hloc")
            nc.vector.tensor_copy(h_loc[:, :], psum_hl[:, :])

            cc_in = dpool.tile([1, N], BF16, tag="cc_in")
            cc_out = dpool.tile([W, N], BF16, tag="cc_out")
            nc.gpsimd.dma_start(cc_in[:, :], h_loc[:, :])
            nc.gpsimd.collective_compute(
                "AllGather",
                mybir.AluOpType.bypass,
                replica_groups=[list(range(W))],
                ins=[cc_in.opt()],
                outs=[cc_out.opt()],
            )
            hall = cpool.tile([W, N], BF16, tag="hall")
            nc.sync.dma_start(hall[:, :], cc_out[:, :])

            # ---- unpack: rank-sum fused with the transpose (PE contracts
            #      over the 8 gathered rank rows), exact f32 1/(C*S1) scale
            #      on the PSUM->SBUF copies ----
            # n-on-partitions hidden for the PE half: psum_t[n, i3] =
            # sum_r hall[r, n]
            psum_t = ppool.tile([128, 2], F32, tag="pt")
            for i3, (off, kk) in enumerate(NCH):
                nc.tensor.matmul(
                    psum_t[0:kk, i3:i3 + 1], hall[:, off:off + kk], ones8[:, :]
                )
            h_nt = cpool.tile([128, 2], BF16, tag="hnt")
            nc.vector.tensor_scalar_mul(h_nt[:, :], psum_t[:, :], 1.0 / (C * S1))

            # partition-broadcast hidden for the DVE half: rank-sum and
            # 128-way broadcast in one matmul (stationary = [8,128] ones)
            psum_r = ppool.tile([128, N], F32, tag="pr")
            nc.tensor.matmul(psum_r[:, :], ones8w[:, :], hall[:, :])
            h_rep = cpool.tile([128, N], BF16, tag="hrep")
            nc.vector.tensor_scalar_mul(h_rep[:, :], psum_r[:, :], 1.0 / (C * S1))

            # ---- GEMM2 PE lane: partial logits (n<256) [p, b]*S2 ----
            psum_l = ppool.tile([128, PEB], F32, tag="pl")
            for b in range(PEB):
                g, bb = divmod(b, W2G)
                for i3, (off, kk) in enumerate(NCH):
                    nc.tensor.matmul(
                        psum_l[:, b:b + 1],
                        w2_sb[(i3, g)][:, bb * 128:(bb + 1) * 128],
                        h_nt[0:kk, i3:i3 + 1],
                        start=(i3 == 0),
                        stop=(i3 == len(NCH) - 1),
                    )

            # ---- DVE lane: the 44-column n-tail of every block via
            #      grouped bf16 mul + one 2x-mode bf16 add + short reduce ----
            lg_q = cpool.tile([128, PEB], F32, tag="lgq")
            h_q = h_rep[:, N2:N].rearrange("p (x n) -> p x n", x=1)
            with nc.allow_low_precision(
                reason="bf16 add-cascade of bf16 products; |sum| << 1"
            ):
                for t, qb0, nb in w2q_sb:
                    scq = spool.tile([128, nb * NQ], BF16, tag="scq")
                    q3 = scq[:, :].rearrange("p (b n) -> p b n", b=nb)
                    nc.vector.tensor_mul(
                        q3,
                        t[:, 0:nb * NQ].rearrange("p (b n) -> p b n", b=nb),
                        h_q.broadcast_to([128, nb, NQ]),
                    )
                    scq2 = spool.tile([128, nb * 22], BF16, tag="scq2")
                    q2v = scq2[:, :].rearrange("p (b n) -> p b n", b=nb)
                    nc.vector.tensor_add(q2v, q3[:, :, 0:22], q3[:, :, 22:44])
                    nc.vector.tensor_reduce(
                        lg_q[:, qb0:qb0 + nb],
                        q2v,
                        mybir.AxisListType.X,
                        mybir.AluOpType.add,
                    )

            # combine the PE psum (n<256) with the DVE n-tail
            e_pe = cpool.tile([128, PEB], F32, tag="epe")
            nc.vector.tensor_add(e_pe[:, :], psum_l[:, :], lg_q[:, :])

            # ---- softmax ----
            e_sb = cpool.tile([128, NB], F32, tag="esb")
            esum2 = cpool.tile([128, 1], F32, tag="esum2")
            nc.scalar.activation(
                e_sb[:, :],
                e_pe[:, :],
                mybir.ActivationFunctionType.Exp,
                scale=1.0 / S2,
                accum_out=esum2[:, 0:1],
            )
            psum_s = ppool.tile([1, 1], F32, tag="ps")
            nc.tensor.matmul(psum_s[:, :], ones128[:, :], esum2[:, :])

            ls = cpool.tile([1, 1], F32, tag="ls")
            nc.vector.tensor_copy(ls[:, :], psum_s[:, :])

            cc2_in = dpool.tile([1, 1], F32, tag="cc2_in")
            cc2_out = dpool.tile([1, W], F32, tag="cc2_out")
            nc.gpsimd.dma_start(cc2_in[:, :], ls[:, :])
            nc.gpsimd.collective_compute(
                "AllGather",
                mybir.AluOpType.bypass,
                replica_groups=[list(range(W))],
                ins=[cc2_in.opt()],
                outs=[cc2_out.opt()],
            )
            # broadcast-read the 8 rank sums to every partition, then the
            # whole normalize stays on DVE (no cross-engine hops)
            sall = cpool.tile([128, W], F32, tag="sall")
            nc.gpsimd.dma_start(sall[:, :], cc2_out[:, :].broadcast_to([128, W]))
            tsum = cpool.tile([128, 1], F32, tag="tsum")
            nc.vector.tensor_reduce(
                tsum[:, :], sall[:, :], mybir.AxisListType.X, mybir.AluOpType.add
            )
            rb = cpool.tile([128, 1], F32, tag="rb")
            nc.vector.reciprocal(rb[:, :], tsum[:, :])

            # scale+store in two halves on the idle SP ring so the second
            # half's scale overlaps the first half's DMA
            y_sb = cpool.tile([128, NB], F32, tag="ysb")
            HB = NB // 2
            nc.vector.tensor_scalar_mul(y_sb[:, 0:HB], e_sb[:, 0:HB], rb[:, :])
            nc.sync.dma_start(y_out[:, 0:HB], y_sb[:, 0:HB])
            nc.vector.tensor_scalar_mul(y_sb[:, HB:NB], e_sb[:, HB:NB], rb[:, :])
            nc.sync.dma_start(y_out[:, HB:NB], y_sb[:, HB:NB])

            # shared-memory probe readback (best effort, end of stream)
            if PROBE:
                dbg_sb = cpool.tile([1, 1], mybir.dt.int32, tag="dbgsb")
                nc.gpsimd.dma_start(dbg_sb[:, :], shp[:, :])
                nc.gpsimd.dma_start(dbg_out[:, :], dbg_sb[:, :])

    _split_multi_waits(nc)
    return nc


_NC_CACHE = None


def _get_nc():
    global _NC_CACHE
    if _NC_CACHE is None:
        _NC_CACHE = build_kernel()
    return _NC_CACHE


def _prep_inputs(context_words, W_in, W_out):
    """Host-side shard + layout prep (pure data movement + dtype cast)."""
    in_maps = []
    for r in range(W):
        v0 = r * VL
        # ctxp[p, j*CP + c] = ctx[c, 128j + p], zero-padded c in [10, 16)
        ctx_s = np.asarray(context_words[:, v0:v0 + VL], dtype=np.float32)
        ctxp = np.zeros((128, NJ, CP), dtype=NP_FP8)
        ctxp[:, :, 0:C] = ctx_s.reshape(C, NJ, 128).transpose(2, 1, 0)
        ctxp = np.ascontiguousarray(ctxp.reshape(128, NJ * CP))
        # w1t[p, j*NP + n] = W_in[n, v0 + 128j + p]*S1, zero-padded n in [300, 304)
        w1s = (W_in[:, v0:v0 + VL].astype(np.float32) * S1).T
        w1t = np.zeros((128, NJ, NP), dtype=NP_FP8)
        w1t[:, :, 0:N] = w1s.reshape(NJ, 128, N).transpose(1, 0, 2)
        w1t = np.ascontiguousarray(w1t.reshape(128, NJ * NP))
        # ws[p, b, n] = W_out[v0 + 125p + b, n]
        ws = np.asarray(W_out[v0:v0 + VL, :], dtype=np.float32).reshape(128, NB, N)
        # PE lane: w2p[n, 128b + p] = ws[p, b, n]*S2, b < PEB, n < 256
        w2p = np.ascontiguousarray(
            (ws[:, :PEB, :N2] * S2).transpose(2, 1, 0).reshape(N2, PEB * 128)
        ).astype(NP_FP8)
        # n-tail of the PE-lane blocks (DVE): w2q[p, b*NQ+q] =
        # ws[p, b, 256+q]*S2
        w2q = np.ascontiguousarray(
            (ws[:, :PEB, N2:] * S2).reshape(128, PEB * NQ)
        ).astype(NP_BF16)
        in_maps.append({"ctxp": ctxp, "w1t": w1t, "w2p": w2p, "w2q": w2q})
    return in_maps


def kernel(context_words, W_in, W_out):
    nc = _get_nc()
    in_maps = _prep_inputs(context_words, W_in, W_out)
    res = run_bass_kernel_spmd(nc, in_maps, list(range(W)))
    # y[p, b] on core r = prob[r*VL + 125*p + b]
    return np.concatenate(
        [np.asarray(res.results[r]["y"], dtype=np.float32).reshape(VL) for r in range(W)]
    )


# revision 40
# speedup vs baseline: 1.1263x; 1.1263x over previous
"""CBOW (nn_CBOW_88991722373900) Trainium2 kernel.

Full-input contract: kernel(context_words[10,128000] f32, W_in[300,128000] f32,
W_out[128000,300] f32) -> softmax probabilities [128000] f32.

Strategy (8-way tensor/model parallel over the vocab dim V):
  - shard V into 8 chunks of 16000; each core holds its slice of both weight
    matrices, cast on host to fp8e4 (e4m3, +-240 range) with power-of-two
    scales folded out later - memory-bound problem, quarters HBM traffic vs
    f32 and halves it vs bf16
  - GEMM1: partial hidden[10,300] accumulated in PSUM over 125 v-chunks,
    fp8 DoubleRow perf mode (2 v-chunks per matmul, 2x PE throughput);
    C and N padded to 16/304 for the DoubleRow 16B step constraint
  - C-reduce (ones-matmul) -> AllGather(300 bf16) -> rank-sum (ones-matmul),
    exact f32 (1/(C*S1)) on the PSUM->SBUF copies -> full hidden
  - GEMM2 split across two engines in parallel:
      PE half  (v-blocks 0..74):   fp8 W_out col-blocks stationary (FWL),
               hidden col moving -> PSUM [128, 75]
      DVE half (v-blocks 75..124): grouped bf16 tensor_mul against a
               partition-broadcast hidden + segmented bf16 tensor_reduce
               (all-2-byte operands keep the DVE 2x mode) -> SBUF [128, 50]
    v mapped so partition p holds contiguous v = 125*p + b
  - softmax: exp on ScalarE with the 2^-12 w2-scale folded into the
    activation scale (no max subtraction: |logit| << 1), local sum via
    ones-matmul, AllGather(4B) for the global denominator, scale, DMA out
"""

import numpy as np
import ml_dtypes

import concourse.bass as bass
import concourse.mybir as mybir
from concourse import tile
from concourse.bass_utils import run_bass_kernel_spmd
from concourse.vector_clock import ScopedClock, VectorClock

V = 128000
N = 300
C = 10
W = 8              # cores
VL = V // W        # 16000 vocab per core
NJ = VL // 128     # 125 v-chunks for GEMM1
NB = VL // 128     # 125 v-blocks for GEMM2
CP = 16            # C padded for DoubleRow 16B step constraint
NP = 304           # N padded (must be mult of 16)
NCH = [(0, 128), (128, 128)]  # n-chunks for GEMM2 PE (n<256; 44-tail on DVE)
N2 = 256           # n columns handled by the PE lane
NQ = N - N2        # 44-column n-tail of the PE-lane blocks, done on DVE
PEB = 125          # all v-blocks on the PE lane (2-chunk matvec)
W2G = 25           # PE-lane v-blocks per w2 SBUF tile group
NG2 = (PEB + W2G - 1) // W2G
QG = 42            # PE-lane blocks per n-tail SBUF tile group
NGQ = (PEB + QG - 1) // QG

S1 = float(2 ** 16)   # host scale on W_in  (max .0028*65536 = 184 < 240)
S2 = float(2 ** 12)   # host scale on W_out (max .0577*4096 = 236 < 240)

BF16 = mybir.dt.bfloat16
F32 = mybir.dt.float32
FP8 = mybir.dt.float8e4
NP_BF16 = ml_dtypes.bfloat16
NP_FP8 = ml_dtypes.float8_e4m3

# w1/ctx chunk group sizes (even prefix sums so DoubleRow pairs never span
# a group boundary); first groups small so GEMM1 starts early
PROBE = False  # shared-memory probe (failed NEFF compile; keep off)

W1_GROUPS = [2, 4, 6, 8] + [10] * 10 + [5]
assert sum(W1_GROUPS) == NJ


def _patched_drain_and_barrier(self, tick_clock, wait_clock):
    """Tail-drain waits split into 1-wait NOPs (this walrus build's CTRL
    instructions only encode a single sync wait), and the trailing
    all-engine barrier after the semaphore clear dropped: engines halt
    right after, and the runtime only starts a new execution once every
    engine has halted."""
    vc = tick_clock.global_clock
    procs = [(p, vc[p]) for p in range(len(vc)) if vc[p] > 0]
    for i, (p, t) in enumerate(procs):
        pvc = VectorClock([0] * len(vc))
        pvc.require_at_least(p, t)
        nop_inst = self.nc.sync.nop(nofuse=True, hint=f"tail_wait_{i}")
        wait_clock.add_sem_waits(nop_inst.ins, ScopedClock({None: pvc}))
    self.nc.sync.drain()
    self.nc.all_engine_barrier(sem_only=True)
    assert self.sems is not None
    popped = self.nc._tile_sem_poison_stack.pop()
    assert popped is self._sem_poison
    self.nc.clear_and_free_semaphores(list(self.sems.allocated().values()))


tile.TileContext._drain_and_barrier = _patched_drain_and_barrier


def _split_multi_waits(nc):
    """This walrus build encodes at most ONE sync wait per instruction. Hoist
    excess waits onto same-engine NoOps inserted immediately before."""
    import bass_rust

    ctr = [0]

    def make_nop(engine, wait):
        ctr[0] += 1
        nop = mybir.InstNoOp(name=f"I-wsplit{ctr[0]}", engine=engine)
        nop.bass_nofuse = True
        nop.sync_info = bass_rust.SyncInfo(on_wait=[wait], on_update=[])
        nc.register_instruction(nop, overwrite=True)
        return nop

    for bb in nc.main_func.blocks:
        out = []
        for ins in bb.instructions:
            si = ins.sync_info
            if si is not None and si.on_wait and len(si.on_wait) > 1:
                waits = list(si.on_wait)
                for w in waits[:-1]:
                    out.append(make_nop(ins.engine, w))
                ins.sync_info = bass_rust.SyncInfo(
                    on_wait=[waits[-1]], on_update=list(si.on_update)
                )
            out.append(ins)
        bb.instructions = out


def build_kernel():
    nc = bass.Bass()

    ctxp = nc.dram_tensor("ctxp", [128, NJ * CP], FP8, kind="ExternalInput")
    # w1t packed partition-major on host: w1t[p, j*NP + n] = W_in[n, v0+128j+p]*S1
    w1t = nc.dram_tensor("w1t", [128, NJ * NP], FP8, kind="ExternalInput")
    # w2p: PE lane, w2p[n, 128b+p] = W_out[v0+125p+b, n]*S2, b<PEB, n<256
    w2p = nc.dram_tensor("w2p", [N2, PEB * 128], FP8, kind="ExternalInput")
    # w2q: n-tail of the PE-lane blocks (DVE), w2q[p, b*NQ+q] =
    #      W_out[v0+125p+b, 256+q]*S2
    w2q = nc.dram_tensor("w2q", [128, PEB * NQ], BF16, kind="ExternalInput")
    y_out = nc.dram_tensor("y", [128, NB], F32, kind="ExternalOutput")
    if PROBE:
        shp = nc.dram_tensor("shp", [1, 1], mybir.dt.int32, addr_space="Shared")
        dbg_out = nc.dram_tensor("dbg", [1, 1], mybir.dt.int32, kind="ExternalOutput")

    with tile.TileContext(nc) as tc:
        with (
            tc.tile_pool(name="const", bufs=1) as cpool,
            tc.tile_pool(name="scr", bufs=2) as spool,
            tc.tile_pool(name="psum", bufs=1, space="PSUM") as ppool,
            tc.tile_pool(name="dram", bufs=1, space="DRAM") as dpool,
        ):
            # ---- shared-memory probe: all cores race-write their rank+10
            #      into one Shared DRAM word; read back at the end. Shared
            #      HBM => every core sees the same winner; per-core HBM =>
            #      each core reads back its own rank. ----
            if PROBE:
                rank = nc.gpsimd.partition_id()
                rk_sb = cpool.tile([1, 1], mybir.dt.int32, tag="rk")
                rk10 = nc.gpsimd.alloc_register("rk10")
                nc.gpsimd.reg_add(rk10, rank, 10)
                nc.gpsimd.reg_save(rk_sb[:, :], rk10)
                nc.gpsimd.dma_start(shp[:, :], rk_sb[:, :])

            # ---- ctx staged in two pieces so the first GEMM1 matmul only
            #      waits on a 4KB transfer; the large piece goes first on
            #      the fast SP HWDGE ring ----
            CTX_SPLIT = W1_GROUPS[0]  # chunks in the first piece
            ctx_a = cpool.tile([128, CTX_SPLIT * CP], FP8, tag="ctxa")
            nc.gpsimd.dma_start(ctx_a[:, :], ctxp[:, 0:CTX_SPLIT * CP])
            ctx_b = cpool.tile([128, (NJ - CTX_SPLIT) * CP], FP8, tag="ctxb")
            nc.sync.dma_start(ctx_b[:, :], ctxp[:, CTX_SPLIT * CP:])

            def ctx_slice(j, nchunks):
                if j < CTX_SPLIT:
                    return ctx_a[:, j * CP:(j + nchunks) * CP]
                jo = j - CTX_SPLIT
                return ctx_b[:, jo * CP:(jo + nchunks) * CP]

            ones10 = cpool.tile([C, 1], BF16, tag="ones10")
            nc.vector.memset(ones10[:, :], 1.0)
            ones8 = cpool.tile([W, 1], BF16, tag="ones8")
            nc.vector.memset(ones8[:, :], 1.0)
            onescol = cpool.tile([1, 128], F32, tag="onescol")
            nc.vector.memset(onescol[:, :], 1.0)
            ones128 = cpool.tile([128, 1], F32, tag="ones128")
            nc.vector.memset(ones128[:, :], 1.0)
            ones8w = cpool.tile([W, 128], BF16, tag="ones8w")
            nc.vector.memset(ones8w[:, :], 1.0)

            # ---- w1 stream, alternating HWDGE rings (SP / ACT) ----
            rings = [nc.sync, nc.scalar]
            ri = 0
            w1_sb = []
            j0 = 0
            for g, nj in enumerate(W1_GROUPS):
                t = cpool.tile([128, nj * NP], FP8, tag=f"w1_{g}")
                rings[ri % 2].dma_start(t[:, :], w1t[:, j0 * NP:(j0 + nj) * NP])
                ri += 1
                w1_sb.append((t, j0, nj))
                j0 += nj

            # w2 streams right behind w1 on the same rings
            w2_sb = {}
            for g in range(NG2):
                b0 = g * W2G
                nb = min(W2G, PEB - b0)
                for i3, (off, kk) in enumerate(NCH):
                    t = cpool.tile([kk, nb * 128], FP8, tag=f"w2_{i3}_{g}")
                    rings[ri % 2].dma_start(
                        t[:, :], w2p[off:off + kk, b0 * 128:(b0 + nb) * 128]
                    )
                    ri += 1
                    w2_sb[(i3, g)] = t
            w2q_sb = []
            for g in range(NGQ):
                qb0 = g * QG
                nb = min(QG, PEB - qb0)
                t = cpool.tile([128, nb * NQ], BF16, tag=f"w2q_{g}")
                rings[ri % 2].dma_start(
                    t[:, :], w2q[:, qb0 * NQ:(qb0 + nb) * NQ]
                )
                ri += 1
                w2q_sb.append((t, qb0, nb))

            # ---- GEMM1: psum_h[c, n] += ctx_chunk^T x w1 tile, fp8
            #      DoubleRow (2 v-chunks of 128 per matmul) ----
            psum_h = ppool.tile([CP, NP], F32, tag="ph")
            for t, j0g, nj in w1_sb:
                lj = 0
                while lj < nj:
                    j = j0g + lj
                    if lj + 2 <= nj and j + 2 <= NJ:
                        lhsT = ctx_slice(j, 2).rearrange(
                            "q (two c) -> q two c", two=2
                        )
                        rhs = t[:, lj * NP:(lj + 2) * NP].rearrange(
                            "q (two n) -> q two n", two=2
                        )
                        nc.tensor.matmul(
                            psum_h[:, :], lhsT, rhs,
                            start=(j == 0), stop=(j + 2 == NJ),
                            perf_mode=mybir.MatmulPerfMode.DoubleRow,
                        )
                        lj += 2
                    else:
                        nc.tensor.matmul(
                            psum_h[:, :],
                            ctx_slice(j, 1),
                            t[:, lj * NP:(lj + 1) * NP],
                            start=(j == 0), stop=(j + 1 == NJ),
                        )
                        lj += 1

            # tiny dummy Exp in the idle window so the activation table is
            # resident before the real softmax exp
            warm = cpool.tile([1, 1], F32, tag="warm")
            nc.vector.memset(warm[:, :], 0.0)
            warm2 = cpool.tile([1, 1], F32, tag="warm2")
            nc.scalar.activation(
                warm2[:, :], warm[:, :], mybir.ActivationFunctionType.Exp
            )

            # ---- local C-reduce -> [1, 300] bf16 (small AllGather payload;
            #      collective latency here is floor-dominated) ----
            h10 = cpool.tile([C, N], BF16, tag="h10")
            nc.vector.tensor_copy(h10[:, :], psum_h[0:C, 0:N])
            psum_hl = ppool.tile([1, N], F32, tag="phl")
            nc.tensor.matmul(psum_hl[:, :], ones10[:, :], h10[:, :])
            h_loc = cpool.tile([1, N], BF16, tag="hloc")
            nc.vector.tensor_copy(h_loc[:, :], psum_hl[:, :])

            cc_in = dpool.tile([1, N], BF16, tag="cc_in")
            cc_out = dpool.tile([W, N], BF16, tag="cc_out")
            nc.gpsimd.dma_start(cc_in[:, :], h_loc[:, :])
            nc.gpsimd.collective_compute(
                "AllGather",
                mybir.AluOpType.bypass,
                replica_groups=[list(range(W))],
                ins=[cc_in.opt()],
                outs=[cc_out.opt()],
            )
            hall = cpool.tile([W, N], BF16, tag="hall")
            nc.sync.dma_start(hall[:, :], cc_out[:, :])

            # ---- unpack: rank-sum fused with the transpose (PE contracts
            #      over the 8 gathered rank rows), exact f32 1/(C*S1) scale
            #      on the PSUM->SBUF copies ----
            # n-on-partitions hidden for the PE half: psum_t[n, i3] =
            # sum_r hall[r, n]
            psum_t = ppool.tile([128, 2], F32, tag="pt")
            for i3, (off, kk) in enumerate(NCH):
                nc.tensor.matmul(
                    psum_t[0:kk, i3:i3 + 1], hall[:, off:off + kk], ones8[:, :]
                )
            h_nt = cpool.tile([128, 2], BF16, tag="hnt")
            nc.vector.tensor_scalar_mul(h_nt[:, :], psum_t[:, :], 1.0 / (C * S1))

            # partition-broadcast hidden for the DVE half: rank-sum and
            # 128-way broadcast in one matmul (stationary = [8,128] ones)
            psum_r = ppool.tile([128, N], F32, tag="pr")
            nc.tensor.matmul(psum_r[:, :], ones8w[:, :], hall[:, :])
            h_rep = cpool.tile([128, N], BF16, tag="hrep")
            nc.vector.tensor_scalar_mul(h_rep[:, :], psum_r[:, :], 1.0 / (C * S1))

            # ---- GEMM2 PE lane: partial logits (n<256) [p, b]*S2 ----
            psum_l = ppool.tile([128, PEB], F32, tag="pl")
            for b in range(PEB):
                g, bb = divmod(b, W2G)
                for i3, (off, kk) in enumerate(NCH):
                    nc.tensor.matmul(
                        psum_l[:, b:b + 1],
                        w2_sb[(i3, g)][:, bb * 128:(bb + 1) * 128],
                        h_nt[0:kk, i3:i3 + 1],
                        start=(i3 == 0),
                        stop=(i3 == len(NCH) - 1),
                    )

            # ---- DVE lane: the 44-column n-tail of every block via
            #      grouped bf16 mul + one 2x-mode bf16 add + short reduce ----
            lg_q = cpool.tile([128, PEB], F32, tag="lgq")
            h_q = h_rep[:, N2:N].rearrange("p (x n) -> p x n", x=1)
            with nc.allow_low_precision(
                reason="bf16 add-cascade of bf16 products; |sum| << 1"
            ):
                for t, qb0, nb in w2q_sb:
                    scq = spool.tile([128, nb * NQ], BF16, tag="scq")
                    q3 = scq[:, :].rearrange("p (b n) -> p b n", b=nb)
                    nc.vector.tensor_mul(
                        q3,
                        t[:, 0:nb * NQ].rearrange("p (b n) -> p b n", b=nb),
                        h_q.broadcast_to([128, nb, NQ]),
                    )
                    scq2 = spool.tile([128, nb * 22], BF16, tag="scq2")
                    q2v = scq2[:, :].rearrange("p (b n) -> p b n", b=nb)
                    nc.vector.tensor_add(q2v, q3[:, :, 0:22], q3[:, :, 22:44])
                    nc.vector.tensor_reduce(
                        lg_q[:, qb0:qb0 + nb],
                        q2v,
                        mybir.AxisListType.X,
                        mybir.AluOpType.add,
                    )

            # combine the PE psum (n<256) with the DVE n-tail
            e_pe = cpool.tile([128, PEB], F32, tag="epe")
            nc.vector.tensor_add(e_pe[:, :], psum_l[:, :], lg_q[:, :])

            # ---- softmax ----
            e_sb = cpool.tile([128, NB], F32, tag="esb")
            esum2 = cpool.tile([128, 1], F32, tag="esum2")
            nc.scalar.activation(
                e_sb[:, :],
                e_pe[:, :],
                mybir.ActivationFunctionType.Exp,
                scale=1.0 / S2,
                accum_out=esum2[:, 0:1],
            )
            psum_s = ppool.tile([1, 1], F32, tag="ps")
            nc.tensor.matmul(psum_s[:, :], ones128[:, :], esum2[:, :])

            ls = cpool.tile([1, 1], F32, tag="ls")
            nc.vector.tensor_copy(ls[:, :], psum_s[:, :])

            cc2_in = dpool.tile([1, 1], F32, tag="cc2_in")
            cc2_out = dpool.tile([1, W], F32, tag="cc2_out")
            nc.gpsimd.dma_start(cc2_in[:, :], ls[:, :])
            nc.gpsimd.collective_compute(
                "AllGather",
                mybir.AluOpType.bypass,
                replica_groups=[list(range(W))],
                ins=[cc2_in.opt()],
                outs=[cc2_out.opt()],
            )
            # [1,8] read -> scalar 1/Z -> broadcast to 128 partitions via a
            # small f32 ones-matmul (a [128,8] broadcast-read DMA costs ~4us)
            s8 = cpool.tile([1, W], F32, tag="s8")
            nc.gpsimd.dma_start(s8[:, :], cc2_out[:, :])
            zs = cpool.tile([1, 1], F32, tag="zs")
            nc.vector.tensor_reduce(
                zs[:, :], s8[:, :], mybir.AxisListType.X, mybir.AluOpType.add
            )
            rz = cpool.tile([1, 1], F32, tag="rz")
            nc.vector.reciprocal(rz[:, :], zs[:, :])
            psum_b = ppool.tile([128, 1], F32, tag="pb")
            nc.tensor.matmul(psum_b[:, :], onescol[:, :], rz[:, :])
            rb = psum_b

            # scale+store in two halves on the idle SP ring so the second
            # half's scale overlaps the first half's DMA
            y_sb = cpool.tile([128, NB], F32, tag="ysb")
            HB = NB // 2
            nc.vector.tensor_scalar_mul(y_sb[:, 0:HB], e_sb[:, 0:HB], rb[:, 0:1])
            nc.sync.dma_start(y_out[:, 0:HB], y_sb[:, 0:HB])
            nc.vector.tensor_scalar_mul(y_sb[:, HB:NB], e_sb[:, HB:NB], rb[:, 0:1])
            nc.sync.dma_start(y_out[:, HB:NB], y_sb[:, HB:NB])

            # shared-memory probe readback (best effort, end of stream)
            if PROBE:
                dbg_sb = cpool.tile([1, 1], mybir.dt.int32, tag="dbgsb")
                nc.gpsimd.dma_start(dbg_sb[:, :], shp[:, :])
                nc.gpsimd.dma_start(dbg_out[:, :], dbg_sb[:, :])

    _split_multi_waits(nc)
    return nc


_NC_CACHE = None


def _get_nc():
    global _NC_CACHE
    if _NC_CACHE is None:
        _NC_CACHE = build_kernel()
    return _NC_CACHE


def _prep_inputs(context_words, W_in, W_out):
    """Host-side shard + layout prep (pure data movement + dtype cast)."""
    in_maps = []
    for r in range(W):
        v0 = r * VL
        # ctxp[p, j*CP + c] = ctx[c, 128j + p], zero-padded c in [10, 16)
        ctx_s = np.asarray(context_words[:, v0:v0 + VL], dtype=np.float32)
        ctxp = np.zeros((128, NJ, CP), dtype=NP_FP8)
        ctxp[:, :, 0:C] = ctx_s.reshape(C, NJ, 128).transpose(2, 1, 0)
        ctxp = np.ascontiguousarray(ctxp.reshape(128, NJ * CP))
        # w1t[p, j*NP + n] = W_in[n, v0 + 128j + p]*S1, zero-padded n in [300, 304)
        w1s = (W_in[:, v0:v0 + VL].astype(np.float32) * S1).T
        w1t = np.zeros((128, NJ, NP), dtype=NP_FP8)
        w1t[:, :, 0:N] = w1s.reshape(NJ, 128, N).transpose(1, 0, 2)
        w1t = np.ascontiguousarray(w1t.reshape(128, NJ * NP))
        # ws[p, b, n] = W_out[v0 + 125p + b, n]
        ws = np.asarray(W_out[v0:v0 + VL, :], dtype=np.float32).reshape(128, NB, N)
        # PE lane: w2p[n, 128b + p] = ws[p, b, n]*S2, b < PEB, n < 256
        w2p = np.ascontiguousarray(
            (ws[:, :PEB, :N2] * S2).transpose(2, 1, 0).reshape(N2, PEB * 128)
        ).astype(NP_FP8)
        # n-tail of the PE-lane blocks (DVE): w2q[p, b*NQ+q] =
        # ws[p, b, 256+q]*S2
        w2q = np.ascontiguousarray(
            (ws[:, :PEB, N2:] * S2).reshape(128, PEB * NQ)
        ).astype(NP_BF16)
        in_maps.append({"ctxp": ctxp, "w1t": w1t, "w2p": w2p, "w2q": w2q})
    return in_maps


def kernel(context_words, W_in, W_out):
    nc = _get_nc()
    in_maps = _prep_inputs(context_words, W_in, W_out)
    res = run_bass_kernel_spmd(nc, in_maps, list(range(W)))
    # y[p, b] on core r = prob[r*VL + 125*p + b]
    return np.concatenate(
        [np.asarray(res.results[r]["y"], dtype=np.float32).reshape(VL) for r in range(W)]
    )
